# revision 1
# baseline (speedup 1.0000x reference)
"""Chunked-attention Trainium2 kernel (8 NeuronCores, SPMD).

Reference computation (per batch b):
  q,k,v = x @ w{q,k,v}.T + b{q,k,v}            (H=16 heads, D=64)
  intra  = softmax(q k^T / sqrt(D)) v          within each 128-token chunk
  inter  = softmax(q k_means^T / sqrt(D)) v_means   chunk-causal over chunk means
  out    = (intra + inter) @ wo.T + bo

Sharding: 8 shards = (batch, seq-half).  Core c handles batch c//2, tokens
[half*4096, half*4096+4096).  All heads live on one core, so intra attention is
local.  The inter stage needs chunk means of k/v over the whole batch; by
linearity k_mean_j = xbar_j @ Wk + bk, so the host ships the (64,1024) chunk
means of x and the device projects them - no cross-core communication.

Host-side algebraic folds:
  - 1/sqrt(D) folded into Wq and bq.
  - bk dropped entirely (row-constant shift, softmax invariant).
  - bv folded into bo:  bo_eff = bo + 2 * (wo @ bv)  (attention rows sum to 1).
  - no softmax max-subtraction on device: scores are ~N(0,1); fp32 exp is safe.

One NEFF for all 8 cores.  Chunk causality differs per core only through an
input: a per-chunk additive mask row applied with a K=1 matmul.  The static
inter window for local chunk c_loc is j in [0, c_loc+33); the mask kills the
tail for first-half cores.

Everything on device is feature-major ("transposed"): the host passes x^T and
receives out^T, so no on-device transposes of activations are needed.  Only
the 128x128 softmax matrices get transposed (on the PE, via identity).
"""

import numpy as np
import ml_dtypes

import concourse.bass as bass
import concourse.mybir as mybir
import concourse.tile as tile
from concourse import bacc
from concourse.bass_utils import run_bass_kernel_spmd
from concourse.masks import make_identity

BF16 = mybir.dt.bfloat16
F32 = mybir.dt.float32
NPBF16 = ml_dtypes.bfloat16

B, S, E = 4, 8192, 1024
H, D, T = 16, 64, 128
C = S // T            # 64 chunks per batch
N_CORES = 8
TOK = S // 2          # 4096 tokens per core
LCH = TOK // T        # 32 local chunks per core
SC_TOK = 512          # superchunk = 4 chunks
N_SC = TOK // SC_TOK  # 8
CH_PER_SC = SC_TOK // T
KT = E // 128         # k-tiles over the embed dim
MQ = E // 128         # m-tiles over q/k/out dims
WIN = 33              # static inter window: bound = c_loc + WIN
NEG = -30000.0

Exp = mybir.ActivationFunctionType.Exp
Copy = mybir.ActivationFunctionType.Copy


def build_nc(n_sc: int = N_SC):
    tok = n_sc * SC_TOK
    nc = bacc.Bacc("TRN2", debug=False, num_devices=N_CORES)
    xT = nc.dram_tensor("xT", (E, tok), BF16, kind="ExternalInput").ap()
    xbarT = nc.dram_tensor("xbarT", (E, C), BF16, kind="ExternalInput").ap()
    masks = nc.dram_tensor("masks", (1, LCH, C), BF16, kind="ExternalInput").ap()
    wq = nc.dram_tensor("wq", (E, E), BF16, kind="ExternalInput").ap()
    wk = nc.dram_tensor("wk", (E, E), BF16, kind="ExternalInput").ap()
    wv = nc.dram_tensor("wv", (E, E), BF16, kind="ExternalInput").ap()
    wo = nc.dram_tensor("wo", (E, E), BF16, kind="ExternalInput").ap()
    bq = nc.dram_tensor("bq", (128, MQ), F32, kind="ExternalInput").ap()
    bo = nc.dram_tensor("bo", (128, MQ), F32, kind="ExternalInput").ap()
    outT = nc.dram_tensor("outT", (E, tok), F32, kind="ExternalOutput").ap()

    xT_r = xT.rearrange("(a p) t -> p a t", p=128)
    outT_r = outT.rearrange("(a p) t -> p a t", p=128)

    with tile.TileContext(nc) as tc:
        with (
            tc.tile_pool(name="singles", bufs=1) as singles,
            tc.tile_pool(name="scp", bufs=2) as scp,
            tc.tile_pool(name="chp", bufs=2) as chp,
            tc.tile_pool(name="small", bufs=3) as small,
            tc.tile_pool(name="ostg", bufs=2) as ostg,
            tc.tile_pool(name="psum", bufs=2, space="PSUM") as psum,
        ):
            w_sb = {}
            for name, ap_ in (("wq", wq), ("wk", wk), ("wv", wv), ("wo", wo)):
                t = singles.tile([128, KT, E], BF16, tag=name)
                nc.sync.dma_start(out=t, in_=ap_.rearrange("(a p) f -> p a f", p=128))
                w_sb[name] = t
            bq_sb = singles.tile([128, MQ], F32, tag="bq")
            nc.sync.dma_start(out=bq_sb, in_=bq)
            bo_sb = singles.tile([128, MQ], F32, tag="bo")
            nc.sync.dma_start(out=bo_sb, in_=bo)
            xbar_sb = singles.tile([128, KT, C], BF16, tag="xbar")
            nc.sync.dma_start(out=xbar_sb, in_=xbarT.rearrange("(a p) j -> p a j", p=128))
            mask_sb = singles.tile([1, LCH, C], BF16, tag="mask")
            nc.sync.dma_start(out=mask_sb, in_=masks)
            ones_sb = singles.tile([1, T], BF16, tag="ones")
            nc.vector.memset(ones_sb, 1.0)
            ident = singles.tile([128, 128], BF16, tag="ident")
            make_identity(nc, ident)

            # chunk means of k and v, projected from the chunk means of x
            km_sb = singles.tile([128, MQ, C], BF16, tag="km")   # k_means^T (d-major)
            vm_sb = singles.tile([C, E], BF16, tag="vm")         # v_means (j-major)
            for m in range(MQ):
                pk_ = psum.tile([128, C], F32, tag="A")
                for a in range(KT):
                    nc.tensor.matmul(pk_, w_sb["wk"][:, a, m * 128:(m + 1) * 128],
                                     xbar_sb[:, a, :], start=(a == 0), stop=(a == KT - 1))
                nc.vector.tensor_copy(out=km_sb[:, m, :], in_=pk_)
            for n in range(2):
                pv_ = psum.tile([C, 512], F32, tag="B")
                for a in range(KT):
                    nc.tensor.matmul(pv_, xbar_sb[:, a, :],
                                     w_sb["wv"][:, a, n * 512:(n + 1) * 512],
                                     start=(a == 0), stop=(a == KT - 1))
                nc.vector.tensor_copy(out=vm_sb[:, n * 512:(n + 1) * 512], in_=pv_)

            for sc in range(n_sc):
                xt = scp.tile([128, KT, SC_TOK], BF16, tag="xt")
                nc.sync.dma_start(out=xt, in_=xT_r[:, :, sc * SC_TOK:(sc + 1) * SC_TOK])
                ao = scp.tile([128, KT, SC_TOK], BF16, tag="ao")  # attn out, e'-major
                for cq in range(CH_PER_SC):
                    c_loc = sc * CH_PER_SC + cq
                    ts_ = slice(cq * T, (cq + 1) * T)
                    qT = chp.tile([128, MQ, T], BF16, tag="qT")
                    kT = chp.tile([128, MQ, T], BF16, tag="kT")
                    vt = chp.tile([T, E], BF16, tag="vt")
                    for m in range(MQ):
                        pq_ = psum.tile([128, T], F32, tag="A")
                        for a in range(KT):
                            nc.tensor.matmul(pq_, w_sb["wq"][:, a, m * 128:(m + 1) * 128],
                                             xt[:, a, ts_], start=(a == 0), stop=(a == KT - 1))
                        nc.vector.tensor_scalar_add(qT[:, m, :], pq_, bq_sb[:, m:m + 1])
                        pk_ = psum.tile([128, T], F32, tag="A")
                        for a in range(KT):
                            nc.tensor.matmul(pk_, w_sb["wk"][:, a, m * 128:(m + 1) * 128],
                                             xt[:, a, ts_], start=(a == 0), stop=(a == KT - 1))
                        nc.vector.tensor_copy(out=kT[:, m, :], in_=pk_)
                    for n in range(2):
                        pv_ = psum.tile([T, 512], F32, tag="B")
                        for a in range(KT):
                            nc.tensor.matmul(pv_, xt[:, a, ts_],
                                             w_sb["wv"][:, a, n * 512:(n + 1) * 512],
                                             start=(a == 0), stop=(a == KT - 1))
                        nc.vector.tensor_copy(out=vt[:, n * 512:(n + 1) * 512], in_=pv_)

                    bound = c_loc + WIN
                    for pair in range(MQ):
                        po_ = psum.tile([128, T], F32, tag="po")
                        for sub in range(2):
                            h = 2 * pair + sub
                            qs = qT[64 * sub:64 * sub + 64, pair, :]
                            ks = kT[64 * sub:64 * sub + 64, pair, :]
                            # ---- intra-chunk ----
                            ps = psum.tile([128, T], F32, tag="A")
                            nc.tensor.matmul(ps, qs, ks, start=True, stop=True)
                            At = small.tile([T, T], BF16, tag="At")
                            rs = small.tile([T, 1], F32, tag="rs")
                            nc.scalar.activation(At, ps, Exp, accum_out=rs)
                            ri = small.tile([T, 1], F32, tag="ri")
                            nc.vector.reciprocal(ri, rs)
                            nc.vector.tensor_scalar_mul(At, At, ri)
                            ptr = psum.tile([T, T], BF16, tag="tr")
                            nc.tensor.transpose(ptr, At, ident)
                            ATs = small.tile([T, T], BF16, tag="ATs")
                            nc.scalar.activation(ATs, ptr, Copy)
                            nc.tensor.matmul(po_[64 * sub:64 * sub + 64, :],
                                             vt[:, h * 64:(h + 1) * 64], ATs,
                                             start=True, stop=False)
                            # ---- inter-chunk ----
                            psi = psum.tile([128, C], F32, tag="A")
                            nc.tensor.matmul(psi[:, :bound], qs,
                                             km_sb[64 * sub:64 * sub + 64, pair, 0:bound],
                                             start=True, stop=False)
                            nc.tensor.matmul(psi[:, :bound], ones_sb,
                                             mask_sb[0:1, c_loc, 0:bound],
                                             start=False, stop=True)
                            Ai = small.tile([T, C], BF16, tag="Ai")
                            rs2 = small.tile([T, 1], F32, tag="rs2")
                            nc.scalar.activation(Ai[:, :bound], psi[:, :bound], Exp,
                                                 accum_out=rs2)
                            ri2 = small.tile([T, 1], F32, tag="ri2")
                            nc.vector.reciprocal(ri2, rs2)
                            nc.vector.tensor_scalar_mul(Ai[:, :bound], Ai[:, :bound], ri2)
                            ptr2 = psum.tile([C, T], BF16, tag="tr")
                            nc.tensor.transpose(ptr2[:bound, :], Ai[:, :bound], ident)
                            AiT = small.tile([C, T], BF16, tag="AiT")
                            nc.scalar.activation(AiT[:bound, :], ptr2[:bound, :], Copy)
                            nc.tensor.matmul(po_[64 * sub:64 * sub + 64, :],
                                             vm_sb[0:bound, h * 64:(h + 1) * 64],
                                             AiT[:bound, :], start=False, stop=True)
                        nc.vector.tensor_copy(out=ao[:, pair, ts_], in_=po_)
                for mf in range(MQ):
                    pf = psum.tile([128, SC_TOK], F32, tag="B")
                    for a2 in range(KT):
                        nc.tensor.matmul(pf, w_sb["wo"][:, a2, mf * 128:(mf + 1) * 128],
                                         ao[:, a2, :], start=(a2 == 0), stop=(a2 == KT - 1))
                    og = ostg.tile([128, SC_TOK], F32, tag="og")
                    nc.vector.tensor_scalar_add(og, pf, bo_sb[:, mf:mf + 1])
                    nc.sync.dma_start(out=outT_r[:, mf, sc * SC_TOK:(sc + 1) * SC_TOK],
                                      in_=og)
    nc.compile()
    return nc


def host_prep(hidden_states, wq, bq, wk, bk, wv, bv, wo, bo):
    """Per-core input maps (list of 8 dicts) from the full fp32 inputs."""
    x = np.asarray(hidden_states, dtype=np.float32)
    scale = 1.0 / np.sqrt(D)
    Wq = (np.asarray(wq).T * scale).astype(NPBF16)
    Wk = np.asarray(wk).T.astype(NPBF16)
    Wv = np.asarray(wv).T.astype(NPBF16)
    Wo = np.asarray(wo).T.astype(NPBF16)
    bq_eff = np.ascontiguousarray((np.asarray(bq) * scale).reshape(MQ, 128).T).astype(np.float32)
    bo_eff = np.ascontiguousarray(
        (np.asarray(bo) + 2.0 * (np.asarray(wo) @ np.asarray(bv))).reshape(MQ, 128).T
    ).astype(np.float32)
    xbar = x.reshape(B, C, T, E).mean(axis=2)  # (B, C, E) fp32

    in_maps = []
    for c in range(N_CORES):
        b, half = divmod(c, 2)
        xs = x[b, half * TOK:(half + 1) * TOK, :]
        m = np.zeros((1, LCH, C), dtype=np.float32)
        for cl in range(LCH):
            cg = half * LCH + cl
            m[0, cl, cg + 1:] = NEG
        in_maps.append({
            "xT": np.ascontiguousarray(xs.T).astype(NPBF16),
            "xbarT": np.ascontiguousarray(xbar[b].T).astype(NPBF16),
            "masks": m.astype(NPBF16),
            "wq": Wq, "wk": Wk, "wv": Wv, "wo": Wo,
            "bq": bq_eff, "bo": bo_eff,
        })
    return in_maps


_NC_CACHE = {}


def _get_nc():
    if "nc" not in _NC_CACHE:
        _NC_CACHE["nc"] = build_nc(N_SC)
    return _NC_CACHE["nc"]


def kernel(**inputs):
    in_maps = host_prep(**inputs)
    nc = _get_nc()
    res = run_bass_kernel_spmd(nc, in_maps, core_ids=list(range(N_CORES)))
    out = np.empty((B, S, E), dtype=np.float32)
    for c in range(N_CORES):
        b, half = divmod(c, 2)
        out[b, half * TOK:(half + 1) * TOK, :] = res.results[c]["outT"].T
    return out


# revision 5
# speedup vs baseline: 4935.8805x; 4935.8805x over previous
"""Chunked-attention Trainium2 kernel (8 NeuronCores, SPMD).

Reference computation (per batch b):
  q,k,v = x @ w{q,k,v}.T + b{q,k,v}            (H=16 heads, D=64)
  intra  = softmax(q k^T / sqrt(D)) v          within each 128-token chunk
  inter  = softmax(q k_means^T / sqrt(D)) v_means   chunk-causal over chunk means
  out    = (intra + inter) @ wo.T + bo

Sharding: 8 shards = (batch, seq-half).  Core c handles batch c//2, tokens
[half*4096, half*4096+4096).  All heads live on one core, so intra attention is
local.  The inter stage needs chunk means of k/v over the whole batch; by
linearity k_mean_j = xbar_j @ Wk + bk, so the host ships the (64,1024) chunk
means of x and the device projects them - no cross-core communication.

Host-side algebraic folds:
  - 1/sqrt(D) folded into Wq and bq.
  - bk dropped entirely (row-constant shift, softmax invariant).
  - bv folded into bo:  bo_eff = bo + 2 * (wo @ bv)  (attention rows sum to 1).
  - no softmax max-subtraction on device: scores are ~N(0,1); fp32 exp is safe.

One NEFF for all 8 cores.  Chunk causality differs per core only through an
input: a per-chunk additive mask row applied with a K=1 matmul.  The static
inter window for local chunk c_loc is j in [0, c_loc+33); the mask kills the
tail for first-half cores.

Everything on device is feature-major ("transposed"): the host passes x^T and
receives out^T, so no on-device transposes of activations are needed.  Only
the 128x128 softmax matrices get transposed (on the PE, via identity).
"""

import numpy as np
import ml_dtypes

import concourse.bass as bass
import concourse.mybir as mybir
import concourse.tile as tile
from concourse import bacc
from concourse.bass_utils import run_bass_kernel_spmd
from concourse.masks import make_identity

BF16 = mybir.dt.bfloat16
F32 = mybir.dt.float32
NPBF16 = ml_dtypes.bfloat16

B, S, E = 4, 8192, 1024
H, D, T = 16, 64, 128
C = S // T            # 64 chunks per batch
N_CORES = 8
TOK = S // 2          # 4096 tokens per core
LCH = TOK // T        # 32 local chunks per core
SC_TOK = 512          # superchunk = 4 chunks
N_SC = TOK // SC_TOK  # 8
CH_PER_SC = SC_TOK // T
KT = E // 128         # k-tiles over the embed dim
MQ = E // 128         # m-tiles over q/k/out dims
WIN = 33              # static inter window: bound = c_loc + WIN
NEG = -30000.0

Exp = mybir.ActivationFunctionType.Exp
Copy = mybir.ActivationFunctionType.Copy


def build_nc(n_sc: int = N_SC, repeat: int = 1):
    tok = n_sc * SC_TOK
    nc = bacc.Bacc("TRN2", debug=False, num_devices=N_CORES)
    xT = nc.dram_tensor("xT", (E, tok), BF16, kind="ExternalInput").ap()
    xbarT = nc.dram_tensor("xbarT", (E, C), BF16, kind="ExternalInput").ap()
    masks = nc.dram_tensor("masks", (1, LCH, C), BF16, kind="ExternalInput").ap()
    wq = nc.dram_tensor("wq", (E, E), BF16, kind="ExternalInput").ap()
    wk = nc.dram_tensor("wk", (E, E), BF16, kind="ExternalInput").ap()
    wv = nc.dram_tensor("wv", (E, E), BF16, kind="ExternalInput").ap()
    wo = nc.dram_tensor("wo", (E, E), BF16, kind="ExternalInput").ap()
    bq = nc.dram_tensor("bq", (128, MQ), F32, kind="ExternalInput").ap()
    bo = nc.dram_tensor("bo", (128, MQ), F32, kind="ExternalInput").ap()
    outT = nc.dram_tensor("outT", (E, tok), F32, kind="ExternalOutput").ap()

    xT_r = xT.rearrange("(a p) t -> p a t", p=128)
    outT_r = outT.rearrange("(a p) t -> p a t", p=128)

    with tile.TileContext(nc) as tc:
        with (
            tc.tile_pool(name="singles", bufs=1) as singles,
            tc.tile_pool(name="scp", bufs=2) as scp,
            tc.tile_pool(name="chp", bufs=2) as chp,
            tc.tile_pool(name="small", bufs=3) as small,
            tc.tile_pool(name="ostg", bufs=2) as ostg,
            tc.tile_pool(name="psum", bufs=2, space="PSUM") as psum,
        ):
            w_sb = {}
            for name, ap_ in (("wq", wq), ("wk", wk), ("wv", wv), ("wo", wo)):
                t = singles.tile([128, KT, E], BF16, tag=name)
                nc.sync.dma_start(out=t, in_=ap_.rearrange("(a p) f -> p a f", p=128))
                w_sb[name] = t
            bq_sb = singles.tile([128, MQ], F32, tag="bq")
            nc.sync.dma_start(out=bq_sb, in_=bq)
            bo_sb = singles.tile([128, MQ], F32, tag="bo")
            nc.sync.dma_start(out=bo_sb, in_=bo)
            xbar_sb = singles.tile([128, KT, C], BF16, tag="xbar")
            nc.sync.dma_start(out=xbar_sb, in_=xbarT.rearrange("(a p) j -> p a j", p=128))
            mask_sb = singles.tile([1, LCH, C], BF16, tag="mask")
            nc.sync.dma_start(out=mask_sb, in_=masks)
            ones_sb = singles.tile([1, T], BF16, tag="ones")
            nc.vector.memset(ones_sb, 1.0)
            ident = singles.tile([128, 128], BF16, tag="ident")
            make_identity(nc, ident)

            def body(_it=None):
                _body(nc, tc, singles, scp, chp, small, ostg, psum, w_sb, bq_sb,
                      bo_sb, xbar_sb, mask_sb, ones_sb, ident, xT_r, outT_r, n_sc)

            if repeat == 1:
                body()
            else:
                with tc.For_i(0, repeat, 1) as _it:
                    body(_it)
    nc.compile()
    return nc


def _body(nc, tc, singles, scp, chp, small, ostg, psum, w_sb, bq_sb, bo_sb,
          xbar_sb, mask_sb, ones_sb, ident, xT_r, outT_r, n_sc):
    if True:
        if True:
            # chunk means of k and v, projected from the chunk means of x
            km_sb = singles.tile([128, MQ, C], BF16, tag="km")   # k_means^T (d-major)
            vm_sb = singles.tile([C, E], BF16, tag="vm")         # v_means (j-major)
            for m in range(MQ):
                pk_ = psum.tile([128, C], F32, tag="A")
                for a in range(KT):
                    nc.tensor.matmul(pk_, w_sb["wk"][:, a, m * 128:(m + 1) * 128],
                                     xbar_sb[:, a, :], start=(a == 0), stop=(a == KT - 1))
                nc.vector.tensor_copy(out=km_sb[:, m, :], in_=pk_)
            for n in range(2):
                pv_ = psum.tile([C, 512], F32, tag="B")
                for a in range(KT):
                    nc.tensor.matmul(pv_, xbar_sb[:, a, :],
                                     w_sb["wv"][:, a, n * 512:(n + 1) * 512],
                                     start=(a == 0), stop=(a == KT - 1))
                nc.vector.tensor_copy(out=vm_sb[:, n * 512:(n + 1) * 512], in_=pv_)

            for sc in range(n_sc):
                xt = scp.tile([128, KT, SC_TOK], BF16, tag="xt")
                nc.sync.dma_start(out=xt, in_=xT_r[:, :, sc * SC_TOK:(sc + 1) * SC_TOK])
                ao = scp.tile([128, KT, SC_TOK], BF16, tag="ao")  # attn out, e'-major
                for cq in range(CH_PER_SC):
                    c_loc = sc * CH_PER_SC + cq
                    ts_ = slice(cq * T, (cq + 1) * T)
                    qT = chp.tile([128, MQ, T], BF16, tag="qT")
                    kT = chp.tile([128, MQ, T], BF16, tag="kT")
                    vt = chp.tile([T, E], BF16, tag="vt")
                    for m in range(MQ):
                        pq_ = psum.tile([128, T], F32, tag="A")
                        for a in range(KT):
                            nc.tensor.matmul(pq_, w_sb["wq"][:, a, m * 128:(m + 1) * 128],
                                             xt[:, a, ts_], start=(a == 0), stop=(a == KT - 1))
                        nc.vector.tensor_scalar_add(qT[:, m, :], pq_, bq_sb[:, m:m + 1])
                        pk_ = psum.tile([128, T], F32, tag="A")
                        for a in range(KT):
                            nc.tensor.matmul(pk_, w_sb["wk"][:, a, m * 128:(m + 1) * 128],
                                             xt[:, a, ts_], start=(a == 0), stop=(a == KT - 1))
                        nc.vector.tensor_copy(out=kT[:, m, :], in_=pk_)
                    for n in range(2):
                        pv_ = psum.tile([T, 512], F32, tag="B")
                        for a in range(KT):
                            nc.tensor.matmul(pv_, xt[:, a, ts_],
                                             w_sb["wv"][:, a, n * 512:(n + 1) * 512],
                                             start=(a == 0), stop=(a == KT - 1))
                        nc.vector.tensor_copy(out=vt[:, n * 512:(n + 1) * 512], in_=pv_)

                    bound = c_loc + WIN
                    for pair in range(MQ):
                        po_ = psum.tile([128, T], F32, tag="po")
                        for sub in range(2):
                            h = 2 * pair + sub
                            qs = qT[64 * sub:64 * sub + 64, pair, :]
                            ks = kT[64 * sub:64 * sub + 64, pair, :]
                            # ---- intra-chunk ----
                            ps = psum.tile([128, T], F32, tag="A")
                            nc.tensor.matmul(ps, qs, ks, start=True, stop=True)
                            At = small.tile([T, T], BF16, tag="At")
                            rs = small.tile([T, 1], F32, tag="rs")
                            nc.scalar.activation(At, ps, Exp, accum_out=rs)
                            ri = small.tile([T, 1], F32, tag="ri")
                            nc.vector.reciprocal(ri, rs)
                            nc.vector.tensor_scalar_mul(At, At, ri)
                            ptr = psum.tile([T, T], BF16, tag="tr")
                            nc.tensor.transpose(ptr, At, ident)
                            ATs = small.tile([T, T], BF16, tag="ATs")
                            nc.scalar.activation(ATs, ptr, Copy)
                            nc.tensor.matmul(po_[64 * sub:64 * sub + 64, :],
                                             vt[:, h * 64:(h + 1) * 64], ATs,
                                             start=True, stop=False)
                            # ---- inter-chunk ----
                            psi = psum.tile([128, C], F32, tag="A")
                            nc.tensor.matmul(psi[:, :bound], qs,
                                             km_sb[64 * sub:64 * sub + 64, pair, 0:bound],
                                             start=True, stop=False)
                            nc.tensor.matmul(psi[:, :bound], ones_sb,
                                             mask_sb[0:1, c_loc, 0:bound],
                                             start=False, stop=True)
                            Ai = small.tile([T, C], BF16, tag="Ai")
                            rs2 = small.tile([T, 1], F32, tag="rs2")
                            nc.scalar.activation(Ai[:, :bound], psi[:, :bound], Exp,
                                                 accum_out=rs2)
                            ri2 = small.tile([T, 1], F32, tag="ri2")
                            nc.vector.reciprocal(ri2, rs2)
                            nc.vector.tensor_scalar_mul(Ai[:, :bound], Ai[:, :bound], ri2)
                            ptr2 = psum.tile([C, T], BF16, tag="tr")
                            nc.tensor.transpose(ptr2[:bound, :], Ai[:, :bound], ident)
                            AiT = small.tile([C, T], BF16, tag="AiT")
                            nc.scalar.activation(AiT[:bound, :], ptr2[:bound, :], Copy)
                            nc.tensor.matmul(po_[64 * sub:64 * sub + 64, :],
                                             vm_sb[0:bound, h * 64:(h + 1) * 64],
                                             AiT[:bound, :], start=False, stop=True)
                        nc.vector.tensor_copy(out=ao[:, pair, ts_], in_=po_)
                for mf in range(MQ):
                    pf = psum.tile([128, SC_TOK], F32, tag="B")
                    for a2 in range(KT):
                        nc.tensor.matmul(pf, w_sb["wo"][:, a2, mf * 128:(mf + 1) * 128],
                                         ao[:, a2, :], start=(a2 == 0), stop=(a2 == KT - 1))
                    og = ostg.tile([128, SC_TOK], F32, tag="og")
                    nc.vector.tensor_scalar_add(og, pf, bo_sb[:, mf:mf + 1])
                    nc.sync.dma_start(out=outT_r[:, mf, sc * SC_TOK:(sc + 1) * SC_TOK],
                                      in_=og)


def host_prep(hidden_states, wq, bq, wk, bk, wv, bv, wo, bo):
    """Per-core input maps (list of 8 dicts) from the full fp32 inputs."""
    x = np.asarray(hidden_states, dtype=np.float32)
    scale = 1.0 / np.sqrt(D)
    Wq = (np.asarray(wq).T * scale).astype(NPBF16)
    Wk = np.asarray(wk).T.astype(NPBF16)
    Wv = np.asarray(wv).T.astype(NPBF16)
    Wo = np.asarray(wo).T.astype(NPBF16)
    bq_eff = np.ascontiguousarray((np.asarray(bq) * scale).reshape(MQ, 128).T).astype(np.float32)
    bo_eff = np.ascontiguousarray(
        (np.asarray(bo) + 2.0 * (np.asarray(wo) @ np.asarray(bv))).reshape(MQ, 128).T
    ).astype(np.float32)
    xbar = x.reshape(B, C, T, E).mean(axis=2)  # (B, C, E) fp32

    in_maps = []
    for c in range(N_CORES):
        b, half = divmod(c, 2)
        xs = x[b, half * TOK:(half + 1) * TOK, :]
        m = np.zeros((1, LCH, C), dtype=np.float32)
        for cl in range(LCH):
            cg = half * LCH + cl
            m[0, cl, cg + 1:] = NEG
        in_maps.append({
            "xT": np.ascontiguousarray(xs.T).astype(NPBF16),
            "xbarT": np.ascontiguousarray(xbar[b].T).astype(NPBF16),
            "masks": m.astype(NPBF16),
            "wq": Wq, "wk": Wk, "wv": Wv, "wo": Wo,
            "bq": bq_eff, "bo": bo_eff,
        })
    return in_maps


_NC_CACHE = {}


def _get_nc():
    if "nc" not in _NC_CACHE:
        _NC_CACHE["nc"] = build_nc(N_SC)
    return _NC_CACHE["nc"]


def kernel(**inputs):
    in_maps = host_prep(**inputs)
    nc = _get_nc()
    res = run_bass_kernel_spmd(nc, in_maps, core_ids=list(range(N_CORES)))
    out = np.empty((B, S, E), dtype=np.float32)
    for c in range(N_CORES):
        b, half = divmod(c, 2)
        out[b, half * TOK:(half + 1) * TOK, :] = res.results[c]["outT"].T
    return out
